# revision 24
# baseline (speedup 1.0000x reference)
"""Trainium2 Bass kernel for sliding-window (W=16) causal attention with
QK-RMSNorm and ALiBi bias.

Problem shape: B=4, S=2048, dim=1024, H=8 heads, D=128, window=16 (causal).

Sharding: sequence-parallel over the 8192 tokens -> 8 cores x 1024 tokens.
Core c handles batch c//2, sequence half c%2. Each core receives its token
chunk transposed ([dim, 1040] = 16 halo tokens + 1024 own tokens) plus full
(pre-transposed) weight matrices, computes the whole attention block for its
tokens locally (no collectives), and returns [1024, 1024].

Perf notes (evidence from neuron-profile traces):
  - LDWEIGHTS (~190-210ns for a 128-col fp32r stationary operand, fixed
    1.2GHz) leaves no slack over the 213ns N=512 matmul stream, and
    dominates every small-N matmul. All matmuls therefore run bf16 x
    bf16 (LDWEIGHTS ~90-107ns); inputs are host-converted and DMA
    straight into BF16 tiles -- no engine staging casts. (fp32r = fp32
    RNE-rounded to 11 mantissa bits; _round_f32r reproduces the grid on
    the host if a higher-precision path is ever needed. Mixed-width
    matmul operands, e.g. f32r x bf16, are rejected by the verifier.)
  - The PE clock halves (HAM K4) after ~3.4us of idleness: warmup
    matmuls cover the initial x/wk DMA, the v-projection is interleaved
    into the early attention iterations, and wo-matmuls of tile t-3 are
    interleaved into tile t's head loop, giving one continuous ~180us
    full-speed window.
  - All 128x128 transposes (q tiles and attention output) go through
    the DMA xbar (dma_start transpose=True): one [128,1024]bf16 ->
    [128,8,128] DMA per tile, entirely off the PE. Ring buffers are
    sized so a slot is never rewritten while a 3-iteration-lagged
    consumer still reads it.
  - The whole 10MB input stream rides the gpsimd SWDGE queue in exact
    consumption order (x1, wk0, x2, wk1, wq, wv, wo): SWDGE drains at
    high HBM priority, and a single consumption-ordered queue avoids
    the cross-queue DMA-semaphore serialization the HWDGE queues
    exhibit. exp tables load late (q-proj slack) on scalar; sync
    carries only qn transposes + output writes, scalar only
    s2/tables/aoT transposes. The Tile scheduler statically reorders
    by (emission priority, modeled readiness) -- program-text
    interleavings are hints, so delivery order must match consumption
    order or Phase B work gets hoisted into Phase A DMA windows.
  - PWO bufs=2 (double-buffered wo PSUM accumulator, PAT shrunk to 3)
    removes per-half-tile outt-copy serialization bubbles.

"""

import ml_dtypes
import numpy as np
from contextlib import ExitStack

import concourse.bacc as bacc
import concourse.bass as bass
import concourse.mybir as mybir
import concourse.tile as tile
from concourse.masks import make_identity
from concourse.bass_utils import run_bass_kernel_spmd

F32 = mybir.dt.float32
F32R = mybir.dt.float32r
BF16 = mybir.dt.bfloat16
AF = mybir.ActivationFunctionType

H = 8          # heads
D = 128        # head dim
DIM = 1024     # model dim
T = 1024       # own tokens per core
HALO = 16      # sliding window length (left)
TK = T + HALO  # k/v tokens per core (halo + own)
NK = DIM // 128  # contraction tiles
NT = T // 128    # query tiles per core
NKB = 9          # ceil(TK/128) v blocks
QW = 512         # weight-stream half width
EPS = 1e-6
N_CORES = 8
N_WARMUP = 20    # dummy matmuls covering the initial x DMA


def _build_nc():
    nc = bacc.Bacc("TRN2", target_bir_lowering=False, debug=False,
                   num_devices=N_CORES)

    xT = nc.dram_tensor("xT", [DIM, TK], BF16, kind="ExternalInput").ap()
    wqT = nc.dram_tensor("wqT", [DIM, DIM], BF16, kind="ExternalInput").ap()
    wkT = nc.dram_tensor("wkT", [DIM, DIM], BF16, kind="ExternalInput").ap()
    wvT = nc.dram_tensor("wvT", [DIM, DIM], BF16, kind="ExternalInput").ap()
    woT = nc.dram_tensor("woT", [DIM, DIM], BF16, kind="ExternalInput").ap()
    expA_d = nc.dram_tensor("expA", [128, H, 128], F32, kind="ExternalInput").ap()
    expA0_d = nc.dram_tensor("expA0", [128, H, 128], F32, kind="ExternalInput").ap()
    expB_d = nc.dram_tensor("expB", [16, H, 32], F32, kind="ExternalInput").ap()
    out_d = nc.dram_tensor("out", [T, DIM], F32, kind="ExternalOutput").ap()
    s2_d = nc.dram_tensor("s2d", [T], F32).ap()  # internal bounce row->col

    with tile.TileContext(nc) as tc, ExitStack() as ctx:
        # ---- resident tensors (whole kernel) ----
        R = ctx.enter_context(tc.tile_pool(name="res", bufs=1))
        qT_sb = R.tile([128, H, T], BF16, tag="qT")
        kT_sb = R.tile([128, H, TK], BF16, tag="kT")
        v_sb = R.tile([128, NKB, H, D + 2], BF16, tag="v")
        woT_sb = R.tile([128, NK, DIM], BF16, tag="woT")

        # -- warmup: keep the PE busy (HAM at K8) from the very start,
        # covering the initial x/wk DMA --
        WU = ctx.enter_context(tc.tile_pool(name="warm", bufs=1))
        junk = WU.tile([128, QW], BF16, tag="junk")
        nc.vector.memset(junk, 0.0)
        with tc.tile_pool(name="ps_warm", bufs=2, space="PSUM") as PSW:
            for i in range(N_WARMUP):
                pwu = PSW.tile([128, QW], F32, tag="warm", name="pwu")
                nc.tensor.matmul(pwu, lhsT=junk[:, 0:128], rhs=junk,
                                 start=True, stop=True)

        C = ctx.enter_context(tc.tile_pool(name="consts", bufs=1))
        ident = C.tile([128, 128], F32, tag="ident")
        make_identity(nc, ident)
        ident_b = C.tile([128, 128], BF16, tag="identb")
        nc.vector.tensor_copy(ident_b, ident)
        ones_f = C.tile([128, 1], F32, tag="ones")
        nc.vector.memset(ones_f, 1.0)
        ones_r = C.tile([128, 1], F32R, tag="onesr")
        nc.vector.tensor_copy(ones_r, ones_f)
        # exp tables (Phase B data) are DMA'd late, tucked into q-proj
        # slack -- 1MB of f32 head-of-line ahead of x/wk was measured to
        # stall k-proj by ~10us
        expA_sb = C.tile([128, H, 128], F32, tag="expA")
        expA0_sb = C.tile([128, H, 128], F32, tag="expA0")
        expB_sb = C.tile([16, H, 32], F32, tag="expB")
        s2T_sb = C.tile([128, NKB], F32, tag="s2T")
        nc.vector.memset(s2T_sb, 0.0)
        invs_sb = C.tile([128, NKB], F32, tag="invs")
        eps_q = C.tile([128, 1], F32, tag="epsq")
        nc.vector.memset(eps_q, EPS)
        eps_k = C.tile([128, 1], F32, tag="epsk")
        nc.vector.memset(eps_k, 128.0 * EPS)

        # ones columns of v (bf16 1.0 is exact)
        nc.vector.tensor_copy(
            v_sb[:, :, :, D:D + 2],
            ones_f.to_broadcast([128, NKB, H, 2]))

        # ================= Phase A: projections =================
        XP = ctx.enter_context(tc.tile_pool(name="xpool", bufs=1))
        WS = ctx.enter_context(tc.tile_pool(name="wstream", bufs=6))
        with (
            tc.tile_pool(name="worka", bufs=2) as WK,
            tc.tile_pool(name="zaccp", bufs=1) as ZA,
            tc.tile_pool(name="sqpool", bufs=2) as SQ,
            tc.tile_pool(name="ps_proj", bufs=4, space="PSUM") as PSA,
            tc.tile_pool(name="ps_tp", bufs=2, space="PSUM") as PTP,
            tc.tile_pool(name="ps_s2", bufs=1, space="PSUM") as PS2,
        ):
            xT_sb = XP.tile([128, NK, TK], BF16, tag="xT")

            wk_qs = [WS.tile([128, NK, QW], BF16, tag="w", name="wk%d" % qi)
                     for qi in range(2)]
            wq_qs = [WS.tile([128, NK, QW], BF16, tag="w", name="wq%d" % qi)
                     for qi in range(2)]
            wv_qs = [WS.tile([128, NK, QW], BF16, tag="w", name="wv%d" % qi)
                     for qi in range(2)]

            # -- input DMA: the whole input stream rides the gpsimd
            # SWDGE queue (measured: it drains at ~190-250GB/s and takes
            # HBM priority over the HWDGE queues) in exact consumption
            # order: x1+wk0 (split for an early k-proj start), x2, wk1,
            # wq, wv, wo. sync then carries only qn/out traffic and
            # scalar only s2/tables/aoT -- minimal per-queue semaphore
            # churn. --
            for half in range(2):
                nc.gpsimd.dma_start(
                    out=xT_sb[:, 4 * half:4 * half + 4, 0:QW],
                    in_=xT[512 * half:512 * half + 512, 0:QW].rearrange(
                        "(kb p) t -> p kb t", p=128))
                nc.gpsimd.dma_start(
                    out=wk_qs[0][:, 4 * half:4 * half + 4, :],
                    in_=wkT[512 * half:512 * half + 512, 0:QW].rearrange(
                        "(kb p) t -> p kb t", p=128))
            nc.gpsimd.dma_start(
                out=xT_sb[:, :, QW:TK],
                in_=xT[:, QW:TK].rearrange("(kb p) t -> p kb t", p=128))
            nc.gpsimd.dma_start(
                out=wk_qs[1],
                in_=wkT[:, QW:2 * QW].rearrange("(kb p) t -> p kb t", p=128))
            for qi in range(2):
                nc.gpsimd.dma_start(
                    out=wq_qs[qi],
                    in_=wqT[:, QW * qi:QW * qi + QW].rearrange(
                        "(kb p) t -> p kb t", p=128))
            for qi in range(2):
                nc.gpsimd.dma_start(
                    out=wv_qs[qi],
                    in_=wvT[:, QW * qi:QW * qi + QW].rearrange(
                        "(kb p) t -> p kb t", p=128))
            nc.gpsimd.dma_start(
                out=woT_sb,
                in_=woT.rearrange("(kb p) c -> p kb c", p=128))

            # ---- k projection (transposed layout) + sum-of-squares ----
            zacc = [ZA.tile([128, QW], F32R, tag="za%d" % ci,
                            name="za%d" % ci) for ci in range(2)]
            for qi in range(2):
                for ci in range(2):
                    for hi in range(4):
                        h = 4 * qi + hi
                        p = PSA.tile([128, QW], F32, tag="proj", name="psk")
                        for k in range(NK):
                            nc.tensor.matmul(
                                p,
                                lhsT=wk_qs[qi][:, k, 128 * hi:128 * hi + 128],
                                rhs=xT_sb[:, k, QW * ci:QW * ci + QW],
                                start=(k == 0), stop=(k == NK - 1))
                        nc.vector.tensor_copy(
                            kT_sb[:, h, QW * ci:QW * ci + QW], p)
                        z2 = SQ.tile([128, QW], F32, tag="sq")
                        nc.scalar.activation(z2, p, AF.Square)
                        if hi == 0 and qi == 0:
                            nc.vector.tensor_copy(zacc[ci], z2)
                        else:
                            nc.vector.tensor_add(zacc[ci], zacc[ci], z2)

            # ---- q projection + RMS norm; transpose via the DMA xbar
            # (one [128,1024]bf16 -> [128,8,128] transpose DMA per tile,
            # entirely off the PE) ----
            for t in range(NT):
                ps = []
                for qi in range(2):
                    p = PSA.tile([128, QW], F32, tag="proj", name="psq")
                    for k in range(NK):
                        nc.tensor.matmul(
                            p,
                            lhsT=xT_sb[:, k, HALO + 128 * t:HALO + 128 * t + 128],
                            rhs=wq_qs[qi][:, k, :],
                            start=(k == 0), stop=(k == NK - 1))
                    ps.append(p)
                sh = []
                for qi in range(2):
                    scr = SQ.tile([128, QW], F32, tag="sq")
                    s1 = WK.tile([128, 1], F32, tag="sh%d" % qi)
                    nc.scalar.activation(scr, ps[qi], AF.Square,
                                         accum_out=s1)
                    sh.append(s1)
                ssum = WK.tile([128, 1], F32, tag="ss")
                nc.vector.tensor_add(ssum, sh[0], sh[1])
                rtmp = WK.tile([128, 1], F32, tag="rt")
                nc.scalar.activation(rtmp, ssum, AF.Sqrt,
                                     bias=eps_q, scale=1.0 / DIM)
                invr = WK.tile([128, 1], F32, tag="ir")
                nc.vector.reciprocal(invr, rtmp)
                qn = WK.tile([128, DIM], BF16, tag="qn", bufs=3)
                for qi in range(2):
                    nc.vector.tensor_scalar_mul(
                        qn[:, QW * qi:QW * qi + QW], ps[qi], invr)
                nc.sync.dma_start(
                    out=qT_sb[:, :, 128 * t:128 * t + 128], in_=qn,
                    transpose=True)
                if t == 4:
                    nc.scalar.dma_start(out=expA_sb, in_=expA_d)
                elif t == 5:
                    nc.scalar.dma_start(out=expA0_sb, in_=expA0_d)
                elif t == 6:
                    nc.scalar.dma_start(out=expB_sb, in_=expB_d)


            # halo k + s2/invs chain, emitted AFTER q-proj: the
            # serial PE->ACT->DVE->DMA->DMA->ACT->DVE chain overlaps
            # q-proj instead of stalling it (invs/kT-halo are only
            # needed at Phase B)
            # halo k: natural [16, 1024] (cheap LDWEIGHTS), then transpose
            khn = WK.tile([16, DIM], BF16, tag="khn", bufs=1)
            s1h = [WK.tile([16, 1], F32, tag="s1h%d" % qi,
                           name="s1h%d" % qi) for qi in range(2)]
            for qi in range(2):
                ph = PSA.tile([128, QW], F32, tag="proj", name="pskh")
                for k in range(NK):
                    nc.tensor.matmul(
                        ph[:16, :],
                        lhsT=xT_sb[:, k, T:T + HALO],
                        rhs=wk_qs[qi][:, k, :],
                        start=(k == 0), stop=(k == NK - 1))
                scr = SQ.tile([128, QW], F32, tag="sq")
                nc.scalar.activation(scr[:16, :], ph[:16, :], AF.Square,
                                     accum_out=s1h[qi])
                nc.vector.tensor_copy(khn[:, QW * qi:QW * qi + QW],
                                      ph[:16, :])
            nc.vector.tensor_add(s2T_sb[0:16, 8:9], s1h[0], s1h[1])
            for h in range(H):
                pt = PTP.tile([128, 128], BF16, tag="tp", name="tph")
                nc.tensor.transpose(pt[:, 0:16],
                                    khn[:, 128 * h:128 * h + 128],
                                    ident_b[0:16, 0:16])
                nc.vector.tensor_copy(kT_sb[:, h, T:T + HALO], pt[:, 0:16])

            # main s2: contract partitions with ones, bounce via DRAM
            s2row = WK.tile([1, QW], F32, tag="s2row", bufs=1)
            for ci in range(2):
                p2 = PS2.tile([1, QW], F32, tag="s2c", name="ps2c")
                nc.tensor.matmul(p2, lhsT=ones_r, rhs=zacc[ci],
                                 start=True, stop=True)
                nc.vector.tensor_copy(s2row, p2)
                nc.scalar.dma_start(
                    out=s2_d[QW * ci:QW * ci + QW].rearrange(
                        "(one t) -> one t", one=1),
                    in_=s2row)
            nc.scalar.dma_start(
                out=s2T_sb[:, 0:8],
                in_=s2_d.rearrange("(kb p) -> p kb", p=128))
            # inv_s = 1/sqrt(s2/8 + 128*eps)  (folds the 1/sqrt(D) scale)
            nc.scalar.activation(invs_sb, s2T_sb, AF.Sqrt,
                                 bias=eps_k, scale=0.125)
            nc.vector.reciprocal(invs_sb, invs_sb)

        # ================= Phase B: attention + output proj =================
        with (
            tc.tile_pool(name="aw", bufs=1) as AW,
            tc.tile_pool(name="workb", bufs=3) as WB,
            tc.tile_pool(name="workb2", bufs=2) as WB2,
            tc.tile_pool(name="ps_sc", bufs=2, space="PSUM") as PSC,
            tc.tile_pool(name="ps_at", bufs=3, space="PSUM") as PAT,
            tc.tile_pool(name="ps_wo", bufs=2, space="PSUM") as PWO,
            tc.tile_pool(name="ps_v", bufs=1, space="PSUM") as PV,
        ):
            # ---- v projection, interleaved into early attention
            # iterations (fills the PE during the exp/mul warm-up and
            # keeps the HAM clock high across the phase transition) ----
            def emit_v_block(kb):
                m = 128 if kb < 8 else 16
                for qi in range(2):
                    p = PV.tile([128, QW], F32, tag="pv", name="pv")
                    for k in range(NK):
                        nc.tensor.matmul(
                            p[:m, :],
                            lhsT=xT_sb[:, k, 128 * kb:128 * kb + m],
                            rhs=wv_qs[qi][:, k, :],
                            start=(k == 0), stop=(k == NK - 1))
                    if qi == 0:
                        nc.vector.tensor_copy(
                            v_sb[:m, kb, 0:4, 0:D],
                            p[:m, :].rearrange("p (h d) -> p h d", h=4))
                    else:
                        nc.scalar.activation(
                            v_sb[:m, kb, 4:8, 0:D],
                            p[:m, :].rearrange("p (h d) -> p h d", h=4),
                            AF.Copy)

            emit_v_block(0)
            emit_v_block(1)
            # attn-weight double buffers: aeA_rb[t%2] holds tile t's A
            # weights (written one tile ahead from the chained score matmul)
            aeA_rb = [AW.tile([128, H, 128], BF16, tag="aeAr%d" % i,
                              name="aeAr%d" % i) for i in range(2)]
            # B-weight ring (cols 0:96 are permanent zeros)
            aeB_rb = [AW.tile([16, 128], BF16, tag="aeBr%d" % i,
                              name="aeBr%d" % i) for i in range(4)]
            for rb in aeB_rb:
                nc.vector.memset(rb, 0.0)

            def emit_block_scores(j, h):
                """Chained scores: M_j = kT[block j].T @ qT[128(j-1),+256).
                Yields tile j's A-weights (into aeA_rb[j%2]) and tile j-1's
                B-weights (returned)."""
                if j == 0:
                    # preamble block: A-part of tile 0 only (cols 128:256
                    # are junk queries, never read). rowmask zeroes the
                    # halo-pad keys on half==0 cores.
                    ps = PSC.tile([128, 256], F32, tag="sc", name="ps0")
                    nc.tensor.matmul(
                        ps,
                        lhsT=kT_sb[:, h, 0:128],
                        rhs=qT_sb[:, h, 0:256],
                        start=True, stop=True)
                    aeAf = WB.tile([128, 128], F32, tag="aeAf", name="aeAf")
                    nc.scalar.activation(aeAf, ps[:, 0:128], AF.Exp,
                                         scale=invs_sb[:, 0:1])
                    nc.vector.tensor_mul(
                        aeA_rb[0][:, h, :], aeAf, expA0_sb[:, h, :])
                    return None
                km = 128 if j < 8 else 16
                qlo = 128 * (j - 1) if j < 8 else 128 * (j - 2)
                bcol = 96 if j < 8 else 224   # B-part column offset
                ps = PSC.tile([128, 256], F32, tag="sc", name="psj")
                nc.tensor.matmul(
                    ps[:km, :],
                    lhsT=kT_sb[:, h, 128 * j:128 * j + km],
                    rhs=qT_sb[:, h, qlo:qlo + 256],
                    start=True, stop=True)
                if j < 8:
                    # A-part of tile j (cols 128:256)
                    aeAf = WB.tile([128, 128], F32, tag="aeAf", name="aeAf")
                    nc.scalar.activation(aeAf, ps[:, 128:256], AF.Exp,
                                         scale=invs_sb[:, j:j + 1])
                    if h % 2 == 0:
                        nc.vector.tensor_mul(
                            aeA_rb[j % 2][:, h, :], aeAf, expA_sb[:, h, :])
                    else:
                        nc.gpsimd.tensor_mul(
                            aeA_rb[j % 2][:, h, :], aeAf, expA_sb[:, h, :])
                # B-part of tile j-1 (rows 0:16, 32 query cols)
                aeBf = WB.tile([16, 32], F32, tag="aeBf", name="aeBf")
                nc.scalar.activation(aeBf, ps[0:16, bcol:bcol + 32],
                                     AF.Exp, scale=invs_sb[0:16, j:j + 1])
                aeB = aeB_rb[h % 4]
                nc.vector.tensor_mul(
                    aeB[:, 96:128], aeBf, expB_sb[:, h, :])
                return aeB

            for h in range(H):
                emit_block_scores(0, h)

            def finish_head(t, h, ao, po, aeB, off):
                hv = h if h < H - 1 else h - 1
                nc.tensor.matmul(po, lhsT=aeB,
                                 rhs=v_sb[0:16, t + 1, hv:hv + 2, :],
                                 start=False, stop=True)
                rinv = WB.tile([128, 1], F32, tag="ri", name="ri")
                nc.vector.reciprocal(rinv, po[:, off + D:off + D + 1])
                nc.vector.tensor_mul(ao[:, 128 * h:128 * h + 128],
                                     po[:, off:off + D],
                                     rinv.to_broadcast([128, D]))

            ao_ring = [None, None, None]  # bf16 attn out, tiles t..t-2
            aoT_ring = [None] * 4     # bf16 transposed (xbar DMA)
            pw_cur = [None]

            def emit_wo(tw, h):
                """Two wo accumulation matmuls for tile tw at step h,
                plus the half copies/DMA at half boundaries."""
                aoT = aoT_ring[tw % 4]
                half = h // 4
                for k in (2 * h % 8, 2 * h % 8 + 1):
                    if k == 0:
                        pw_cur[0] = PWO.tile([128, QW], F32, tag="wo", name="pw")
                    nc.tensor.matmul(
                        pw_cur[0],
                        lhsT=aoT[:, k, :],
                        rhs=woT_sb[:, k, QW * half:QW * half + QW],
                        start=(k == 0), stop=(k == NK - 1))
                    if k == NK - 1:
                        outt = WB2.tile([128, QW], F32, tag="outt")
                        if half == 0:
                            nc.scalar.activation(outt, pw_cur[0], AF.Copy)
                        else:
                            nc.vector.tensor_copy(outt, pw_cur[0])
                        nc.sync.dma_start(
                            out=out_d[128 * tw:128 * tw + 128,
                                      QW * half:QW * half + QW],
                            in_=outt)

            for t in range(NT + 2):
                attn = t < NT
                if attn:
                    ao = WB2.tile([128, DIM], BF16, tag="ao", bufs=3)
                    ao_ring[t % 3] = ao
                    aoT_ring[t % 4] = WB2.tile(
                        [128, NK, 128], BF16, tag="aoT", name="aoT",
                        bufs=4)
                pend = []
                for h in range(H):
                    if h == 4 and attn and t <= NKB - 3:
                        emit_v_block(t + 2)
                    if attn:
                        aeB = emit_block_scores(t + 1, h)
                        hv = h if h < H - 1 else h - 1
                        off = 0 if h < H - 1 else D + 2
                        po = PAT.tile([128, 2 * (D + 2)], F32, tag="at")
                        nc.tensor.matmul(po, lhsT=aeA_rb[t % 2][:, h, :],
                                         rhs=v_sb[:, t, hv:hv + 2, :],
                                         start=True, stop=False)
                        pend.append((h, po, aeB, off))
                        if len(pend) > 2:
                            a = pend.pop(0)
                            finish_head(t, a[0], ao, a[1], a[2], a[3])
                    if 2 <= t:
                        emit_wo(t - 2, h)
                for a in pend:
                    finish_head(t, a[0], ao, a[1], a[2], a[3])
                if attn:
                    # transpose ao(t) via the DMA xbar on the scalar
                    # queue -- never behind the 256KB output writes on
                    # sync (consumed by emit_wo at iteration t+2)
                    nc.scalar.dma_start(out=aoT_ring[t % 4], in_=ao,
                                        transpose=True)

    nc.compile()
    return nc


def _round_f32r(x):
    """RNE to the PE's fp32r grid (11 mantissa bits) -- bit-exact match
    of the engine-side rounding, verified on hardware."""
    u = np.ascontiguousarray(x, np.float32).view(np.uint32).astype(np.uint64)
    half = np.uint64(1 << 11)
    one = np.uint64(1)
    lsb = (u >> np.uint64(12)) & one
    r = ((u + half - one + lsb) & np.uint64(0xFFFFF000)).astype(np.uint32)
    return r.view(np.float32)


def _host_tables():
    slopes = 2.0 ** (-np.arange(1, H + 1, dtype=np.float64))  # [H]
    # A block: keys p (=query start + p), queries n; valid iff 0 <= p-n <= 16
    p = np.arange(128)[:, None]
    n = np.arange(128)[None, :]
    rel = (p - n - 16).astype(np.float64)            # j - i
    validA = (p - n >= 0) & (p - n <= 16)
    expA = np.where(validA[None], np.exp(slopes[:, None, None] * rel[None]), 0.0)
    expA = np.ascontiguousarray(
        expA.transpose(1, 0, 2).astype(np.float32))   # [128, H, 128]
    # B block: keys p' (key idx 128+p'), queries n'' (query 96+n'')
    pp = np.arange(16)[:, None]
    nn = np.arange(32)[None, :]
    relB = (16 + pp - nn).astype(np.float64)
    validB = nn - pp >= 16
    expB = np.where(validB[None], np.exp(slopes[:, None, None] * relB[None]), 0.0)
    expB = np.ascontiguousarray(
        expB.transpose(1, 0, 2).astype(np.float32))   # [16, H, 32]
    # expA with the halo-pad keys (rows 0:16) masked out, for tile 0 on
    # the sequence-start cores (half == 0)
    expA0 = expA.copy()
    expA0[0:16] = 0.0
    return expA, expA0, expB


_CACHE = {}


def make_in_maps(x, wq, wk, wv, wo, q_norm_w, k_norm_w):
    x = np.asarray(x, np.float32)
    expA, expA0, expB = _host_tables()
    BF = ml_dtypes.bfloat16
    # q/k norm weights are ones per the problem spec; fold them into the
    # projection weights (exact when they are ones).
    qnw = np.asarray(q_norm_w, np.float32)
    knw = np.asarray(k_norm_w, np.float32)
    wqT = np.ascontiguousarray(
        np.asarray(wq, np.float32).T * qnw[None, :]).astype(BF)
    wkT = np.ascontiguousarray(
        np.asarray(wk, np.float32).T * knw[None, :]).astype(BF)
    wvT = np.ascontiguousarray(np.asarray(wv, np.float32).T).astype(BF)
    woT = np.ascontiguousarray(np.asarray(wo, np.float32).T).astype(BF)

    in_maps = []
    for c in range(N_CORES):
        b, half = c // 2, c % 2
        start = half * T
        if half == 0:
            chunk = np.concatenate(
                [np.zeros((HALO, DIM), np.float32), x[b, 0:T]], axis=0)
        else:
            chunk = x[b, start - HALO:start + T]
        xT_c = np.ascontiguousarray(chunk.T).astype(BF)  # [dim, TK]
        in_maps.append({
            "xT": xT_c, "wqT": wqT, "wkT": wkT, "wvT": wvT, "woT": woT,
            "expA": expA, "expA0": expA0 if half == 0 else expA,
            "expB": expB,
        })
    return in_maps


def assemble_out(results):
    out = np.empty((4, 2048, DIM), np.float32)
    for c in range(N_CORES):
        b, half = c // 2, c % 2
        out[b, half * T:half * T + T] = results[c]["out"]
    return out


def get_nc():
    if "nc" not in _CACHE:
        _CACHE["nc"] = _build_nc()
    return _CACHE["nc"]


def kernel(x, wq, wk, wv, wo, q_norm_w, k_norm_w):
    assert x.shape == (4, 2048, 1024)
    nc = get_nc()
    in_maps = make_in_maps(x, wq, wk, wv, wo, q_norm_w, k_norm_w)
    res = run_bass_kernel_spmd(nc, in_maps, list(range(N_CORES)))
    return assemble_out(res.results)



# revision 25
# speedup vs baseline: 1.0219x; 1.0219x over previous
"""Trainium2 Bass kernel for sliding-window (W=16) causal attention with
QK-RMSNorm and ALiBi bias.

Problem shape: B=4, S=2048, dim=1024, H=8 heads, D=128, window=16 (causal).

Sharding: sequence-parallel over the 8192 tokens -> 8 cores x 1024 tokens.
Core c handles batch c//2, sequence half c%2. Each core receives its token
chunk transposed ([dim, 1040] = 16 halo tokens + 1024 own tokens) plus full
(pre-transposed) weight matrices, computes the whole attention block for its
tokens locally (no collectives), and returns [1024, 1024].

Perf notes (evidence from neuron-profile traces):
  - LDWEIGHTS (~190-210ns for a 128-col fp32r stationary operand, fixed
    1.2GHz) leaves no slack over the 213ns N=512 matmul stream, and
    dominates every small-N matmul. All matmuls therefore run bf16 x
    bf16 (LDWEIGHTS ~90-107ns); inputs are host-converted and DMA
    straight into BF16 tiles -- no engine staging casts. (fp32r = fp32
    RNE-rounded to 11 mantissa bits; _round_f32r reproduces the grid on
    the host if a higher-precision path is ever needed. Mixed-width
    matmul operands, e.g. f32r x bf16, are rejected by the verifier.)
  - The PE clock halves (HAM K4) after ~3.4us of idleness: warmup
    matmuls cover the initial x/wk DMA, the v-projection is interleaved
    into the early attention iterations, and wo-matmuls of tile t-3 are
    interleaved into tile t's head loop, giving one continuous ~180us
    full-speed window.
  - All 128x128 transposes (q tiles and attention output) go through
    the DMA xbar (dma_start transpose=True): one [128,1024]bf16 ->
    [128,8,128] DMA per tile, entirely off the PE. Ring buffers are
    sized so a slot is never rewritten while a 3-iteration-lagged
    consumer still reads it.
  - The whole 10MB input stream rides the gpsimd SWDGE queue in exact
    consumption order (x1, wk0, x2, wk1, wq, wv, wo): SWDGE drains at
    high HBM priority, and a single consumption-ordered queue avoids
    the cross-queue DMA-semaphore serialization the HWDGE queues
    exhibit. exp tables load late (q-proj slack) on scalar; sync
    carries only qn transposes + output writes, scalar only
    s2/tables/aoT transposes. The Tile scheduler statically reorders
    by (emission priority, modeled readiness) -- program-text
    interleavings are hints, so delivery order must match consumption
    order or Phase B work gets hoisted into Phase A DMA windows.
  - PWO bufs=2 (double-buffered wo PSUM accumulator, PAT shrunk to 3)
    removes per-half-tile outt-copy serialization bubbles.

"""

import ml_dtypes
import numpy as np
from contextlib import ExitStack

import concourse.bacc as bacc
import concourse.bass as bass
import concourse.mybir as mybir
import concourse.tile as tile
from concourse.masks import make_identity
from concourse.bass_utils import run_bass_kernel_spmd

F32 = mybir.dt.float32
F32R = mybir.dt.float32r
BF16 = mybir.dt.bfloat16
AF = mybir.ActivationFunctionType

H = 8          # heads
D = 128        # head dim
DIM = 1024     # model dim
T = 1024       # own tokens per core
HALO = 16      # sliding window length (left)
TK = T + HALO  # k/v tokens per core (halo + own)
NK = DIM // 128  # contraction tiles
NT = T // 128    # query tiles per core
NKB = 9          # ceil(TK/128) v blocks
QW = 512         # weight-stream half width
EPS = 1e-6
N_CORES = 8
N_WARMUP = 20    # dummy matmuls covering the initial x DMA


def _build_nc():
    nc = bacc.Bacc("TRN2", target_bir_lowering=False, debug=False,
                   num_devices=N_CORES)

    xT = nc.dram_tensor("xT", [DIM, TK], BF16, kind="ExternalInput").ap()
    wqT = nc.dram_tensor("wqT", [DIM, DIM], BF16, kind="ExternalInput").ap()
    wkT = nc.dram_tensor("wkT", [DIM, DIM], BF16, kind="ExternalInput").ap()
    wvT = nc.dram_tensor("wvT", [DIM, DIM], BF16, kind="ExternalInput").ap()
    woT = nc.dram_tensor("woT", [DIM, DIM], BF16, kind="ExternalInput").ap()
    expA_d = nc.dram_tensor("expA", [128, H, 128], F32, kind="ExternalInput").ap()
    expA0_d = nc.dram_tensor("expA0", [128, H, 128], F32, kind="ExternalInput").ap()
    expB_d = nc.dram_tensor("expB", [16, H, 32], F32, kind="ExternalInput").ap()
    out_d = nc.dram_tensor("out", [T, DIM], F32, kind="ExternalOutput").ap()
    s2_d = nc.dram_tensor("s2d", [T], F32).ap()  # internal bounce row->col

    with tile.TileContext(nc) as tc, ExitStack() as ctx:
        # ---- resident tensors (whole kernel) ----
        R = ctx.enter_context(tc.tile_pool(name="res", bufs=1))
        qT_sb = R.tile([128, H, T], BF16, tag="qT")
        kT_sb = R.tile([128, H, TK], BF16, tag="kT")
        v_sb = R.tile([128, NKB, H, D + 2], BF16, tag="v")
        woT_sb = R.tile([128, NK, DIM], BF16, tag="woT")

        # -- warmup: keep the PE busy (HAM at K8) from the very start,
        # covering the initial x/wk DMA --
        WU = ctx.enter_context(tc.tile_pool(name="warm", bufs=1))
        junk = WU.tile([128, QW], BF16, tag="junk")
        nc.vector.memset(junk, 0.0)
        with tc.tile_pool(name="ps_warm", bufs=2, space="PSUM") as PSW:
            for i in range(N_WARMUP):
                pwu = PSW.tile([128, QW], F32, tag="warm", name="pwu")
                nc.tensor.matmul(pwu, lhsT=junk[:, 0:128], rhs=junk,
                                 start=True, stop=True)

        C = ctx.enter_context(tc.tile_pool(name="consts", bufs=1))
        ident = C.tile([128, 128], F32, tag="ident")
        make_identity(nc, ident)
        ident_b = C.tile([128, 128], BF16, tag="identb")
        nc.vector.tensor_copy(ident_b, ident)
        ones_f = C.tile([128, 1], F32, tag="ones")
        nc.vector.memset(ones_f, 1.0)
        ones_r = C.tile([128, 1], F32R, tag="onesr")
        nc.vector.tensor_copy(ones_r, ones_f)
        # exp tables (Phase B data) are DMA'd late, tucked into q-proj
        # slack -- 1MB of f32 head-of-line ahead of x/wk was measured to
        # stall k-proj by ~10us
        expA_sb = C.tile([128, H, 128], F32, tag="expA")
        expA0_sb = C.tile([128, H, 128], F32, tag="expA0")
        expB_sb = C.tile([16, H, 32], F32, tag="expB")
        s2T_sb = C.tile([128, NKB], F32, tag="s2T")
        nc.vector.memset(s2T_sb, 0.0)
        invs_sb = C.tile([128, NKB], F32, tag="invs")
        eps_q = C.tile([128, 1], F32, tag="epsq")
        nc.vector.memset(eps_q, EPS)
        eps_k = C.tile([128, 1], F32, tag="epsk")
        nc.vector.memset(eps_k, 128.0 * EPS)

        # ones columns of v (bf16 1.0 is exact)
        nc.vector.tensor_copy(
            v_sb[:, :, :, D:D + 2],
            ones_f.to_broadcast([128, NKB, H, 2]))

        # ================= Phase A: projections =================
        XP = ctx.enter_context(tc.tile_pool(name="xpool", bufs=1))
        WS = ctx.enter_context(tc.tile_pool(name="wstream", bufs=6))
        with (
            tc.tile_pool(name="worka", bufs=2) as WK,
            tc.tile_pool(name="zaccp", bufs=1) as ZA,
            tc.tile_pool(name="sqpool", bufs=2) as SQ,
            tc.tile_pool(name="ps_proj", bufs=4, space="PSUM") as PSA,
            tc.tile_pool(name="ps_tp", bufs=2, space="PSUM") as PTP,
            tc.tile_pool(name="ps_s2", bufs=1, space="PSUM") as PS2,
        ):
            xT_sb = XP.tile([128, NK, TK], BF16, tag="xT")

            wk_qs = [WS.tile([128, NK, QW], BF16, tag="w", name="wk%d" % qi)
                     for qi in range(2)]
            wq_qs = [WS.tile([128, NK, QW], BF16, tag="w", name="wq%d" % qi)
                     for qi in range(2)]
            wv_qs = [WS.tile([128, NK, QW], BF16, tag="w", name="wv%d" % qi)
                     for qi in range(2)]

            # -- input DMA: the whole input stream rides the gpsimd
            # SWDGE queue (measured: it drains at ~190-250GB/s and takes
            # HBM priority over the HWDGE queues) in exact consumption
            # order: x1+wk0 (split for an early k-proj start), x2, wk1,
            # wq, wv, wo. sync then carries only qn/out traffic and
            # scalar only s2/tables/aoT -- minimal per-queue semaphore
            # churn. --
            for half in range(2):
                nc.gpsimd.dma_start(
                    out=xT_sb[:, 4 * half:4 * half + 4, 0:QW],
                    in_=xT[512 * half:512 * half + 512, 0:QW].rearrange(
                        "(kb p) t -> p kb t", p=128))
                nc.gpsimd.dma_start(
                    out=wk_qs[0][:, 4 * half:4 * half + 4, :],
                    in_=wkT[512 * half:512 * half + 512, 0:QW].rearrange(
                        "(kb p) t -> p kb t", p=128))
            nc.gpsimd.dma_start(
                out=xT_sb[:, :, QW:TK],
                in_=xT[:, QW:TK].rearrange("(kb p) t -> p kb t", p=128))
            nc.gpsimd.dma_start(
                out=wk_qs[1],
                in_=wkT[:, QW:2 * QW].rearrange("(kb p) t -> p kb t", p=128))
            for qi in range(2):
                nc.gpsimd.dma_start(
                    out=wq_qs[qi],
                    in_=wqT[:, QW * qi:QW * qi + QW].rearrange(
                        "(kb p) t -> p kb t", p=128))
            for qi in range(2):
                nc.gpsimd.dma_start(
                    out=wv_qs[qi],
                    in_=wvT[:, QW * qi:QW * qi + QW].rearrange(
                        "(kb p) t -> p kb t", p=128))
            nc.gpsimd.dma_start(
                out=woT_sb,
                in_=woT.rearrange("(kb p) c -> p kb c", p=128))

            # ---- k projection (transposed layout) + sum-of-squares ----
            zacc = [ZA.tile([128, QW], F32R, tag="za%d" % ci,
                            name="za%d" % ci) for ci in range(2)]
            for qi in range(2):
                for ci in range(2):
                    for hi in range(4):
                        h = 4 * qi + hi
                        p = PSA.tile([128, QW], F32, tag="proj", name="psk")
                        for k in range(NK):
                            nc.tensor.matmul(
                                p,
                                lhsT=wk_qs[qi][:, k, 128 * hi:128 * hi + 128],
                                rhs=xT_sb[:, k, QW * ci:QW * ci + QW],
                                start=(k == 0), stop=(k == NK - 1))
                        nc.vector.tensor_copy(
                            kT_sb[:, h, QW * ci:QW * ci + QW], p)
                        z2 = SQ.tile([128, QW], F32, tag="sq")
                        nc.scalar.activation(z2, p, AF.Square)
                        if hi == 0 and qi == 0:
                            nc.vector.tensor_copy(zacc[ci], z2)
                        else:
                            nc.vector.tensor_add(zacc[ci], zacc[ci], z2)

            # ---- q projection + RMS norm; transpose via the DMA xbar
            # (one [128,1024]bf16 -> [128,8,128] transpose DMA per tile,
            # entirely off the PE) ----
            for t in range(NT):
                ps = []
                for qi in range(2):
                    p = PSA.tile([128, QW], F32, tag="proj", name="psq")
                    for k in range(NK):
                        nc.tensor.matmul(
                            p,
                            lhsT=xT_sb[:, k, HALO + 128 * t:HALO + 128 * t + 128],
                            rhs=wq_qs[qi][:, k, :],
                            start=(k == 0), stop=(k == NK - 1))
                    ps.append(p)
                sh = []
                for qi in range(2):
                    scr = SQ.tile([128, QW], F32, tag="sq")
                    s1 = WK.tile([128, 1], F32, tag="sh%d" % qi)
                    nc.scalar.activation(scr, ps[qi], AF.Square,
                                         accum_out=s1)
                    sh.append(s1)
                ssum = WK.tile([128, 1], F32, tag="ss")
                nc.vector.tensor_add(ssum, sh[0], sh[1])
                rtmp = WK.tile([128, 1], F32, tag="rt")
                nc.scalar.activation(rtmp, ssum, AF.Sqrt,
                                     bias=eps_q, scale=1.0 / DIM)
                invr = WK.tile([128, 1], F32, tag="ir")
                nc.vector.reciprocal(invr, rtmp)
                qn = WK.tile([128, DIM], BF16, tag="qn", bufs=3)
                for qi in range(2):
                    nc.vector.tensor_scalar_mul(
                        qn[:, QW * qi:QW * qi + QW], ps[qi], invr)
                nc.sync.dma_start(
                    out=qT_sb[:, :, 128 * t:128 * t + 128], in_=qn,
                    transpose=True)
                if t == 4:
                    nc.scalar.dma_start(out=expA_sb, in_=expA_d)
                elif t == 5:
                    nc.scalar.dma_start(out=expA0_sb, in_=expA0_d)
                elif t == 6:
                    nc.scalar.dma_start(out=expB_sb, in_=expB_d)


            # halo k + s2/invs chain, emitted AFTER q-proj: the
            # serial PE->ACT->DVE->DMA->DMA->ACT->DVE chain overlaps
            # q-proj instead of stalling it (invs/kT-halo are only
            # needed at Phase B)
            # halo k: natural [16, 1024] (cheap LDWEIGHTS), then transpose
            khn = WK.tile([16, DIM], BF16, tag="khn", bufs=1)
            s1h = [WK.tile([16, 1], F32, tag="s1h%d" % qi,
                           name="s1h%d" % qi) for qi in range(2)]
            for qi in range(2):
                ph = PSA.tile([128, QW], F32, tag="proj", name="pskh")
                for k in range(NK):
                    nc.tensor.matmul(
                        ph[:16, :],
                        lhsT=xT_sb[:, k, T:T + HALO],
                        rhs=wk_qs[qi][:, k, :],
                        start=(k == 0), stop=(k == NK - 1))
                scr = SQ.tile([128, QW], F32, tag="sq")
                nc.scalar.activation(scr[:16, :], ph[:16, :], AF.Square,
                                     accum_out=s1h[qi])
                nc.vector.tensor_copy(khn[:, QW * qi:QW * qi + QW],
                                      ph[:16, :])
            nc.vector.tensor_add(s2T_sb[0:16, 8:9], s1h[0], s1h[1])
            for h in range(H):
                pt = PTP.tile([128, 128], BF16, tag="tp", name="tph")
                nc.tensor.transpose(pt[:, 0:16],
                                    khn[:, 128 * h:128 * h + 128],
                                    ident_b[0:16, 0:16])
                nc.vector.tensor_copy(kT_sb[:, h, T:T + HALO], pt[:, 0:16])

            # main s2: contract partitions with ones, bounce via DRAM
            s2row = WK.tile([1, QW], F32, tag="s2row", bufs=1)
            for ci in range(2):
                p2 = PS2.tile([1, QW], F32, tag="s2c", name="ps2c")
                nc.tensor.matmul(p2, lhsT=ones_r, rhs=zacc[ci],
                                 start=True, stop=True)
                nc.vector.tensor_copy(s2row, p2)
                nc.scalar.dma_start(
                    out=s2_d[QW * ci:QW * ci + QW].rearrange(
                        "(one t) -> one t", one=1),
                    in_=s2row)
            nc.scalar.dma_start(
                out=s2T_sb[:, 0:8],
                in_=s2_d.rearrange("(kb p) -> p kb", p=128))
            # inv_s = 1/sqrt(s2/8 + 128*eps)  (folds the 1/sqrt(D) scale)
            nc.scalar.activation(invs_sb, s2T_sb, AF.Sqrt,
                                 bias=eps_k, scale=0.125)
            nc.vector.reciprocal(invs_sb, invs_sb)

        # ================= Phase B: attention + output proj =================
        with (
            tc.tile_pool(name="aw", bufs=1) as AW,
            tc.tile_pool(name="workb", bufs=3) as WB,
            tc.tile_pool(name="workb2", bufs=2) as WB2,
            tc.tile_pool(name="ps_sc", bufs=2, space="PSUM") as PSC,
            tc.tile_pool(name="ps_at", bufs=3, space="PSUM") as PAT,
            tc.tile_pool(name="ps_wo", bufs=2, space="PSUM") as PWO,
            tc.tile_pool(name="ps_v", bufs=1, space="PSUM") as PV,
        ):
            # ---- v projection, interleaved into early attention
            # iterations (fills the PE during the exp/mul warm-up and
            # keeps the HAM clock high across the phase transition) ----
            def emit_v_block(kb):
                m = 128 if kb < 8 else 16
                for qi in range(2):
                    p = PV.tile([128, QW], F32, tag="pv", name="pv")
                    for k in range(NK):
                        nc.tensor.matmul(
                            p[:m, :],
                            lhsT=xT_sb[:, k, 128 * kb:128 * kb + m],
                            rhs=wv_qs[qi][:, k, :],
                            start=(k == 0), stop=(k == NK - 1))
                    if qi == 0:
                        nc.vector.tensor_copy(
                            v_sb[:m, kb, 0:4, 0:D],
                            p[:m, :].rearrange("p (h d) -> p h d", h=4))
                    else:
                        nc.scalar.activation(
                            v_sb[:m, kb, 4:8, 0:D],
                            p[:m, :].rearrange("p (h d) -> p h d", h=4),
                            AF.Copy)

            emit_v_block(0)
            emit_v_block(1)
            # attn-weight double buffers: aeA_rb[t%2] holds tile t's A
            # weights (written one tile ahead from the chained score matmul)
            aeA_rb = [AW.tile([128, H, 128], BF16, tag="aeAr%d" % i,
                              name="aeAr%d" % i) for i in range(2)]
            # B-weight ring (cols 0:96 are permanent zeros)
            aeB_rb = [AW.tile([16, 128], BF16, tag="aeBr%d" % i,
                              name="aeBr%d" % i) for i in range(4)]
            for rb in aeB_rb:
                nc.vector.memset(rb, 0.0)

            def emit_block_scores(j, h):
                """Chained scores: M_j = kT[block j].T @ qT[128(j-1),+256).
                Yields tile j's A-weights (into aeA_rb[j%2]) and tile j-1's
                B-weights (returned)."""
                if j == 0:
                    # preamble block: A-part of tile 0 only (cols 128:256
                    # are junk queries, never read). rowmask zeroes the
                    # halo-pad keys on half==0 cores.
                    ps = PSC.tile([128, 256], F32, tag="sc", name="ps0")
                    nc.tensor.matmul(
                        ps,
                        lhsT=kT_sb[:, h, 0:128],
                        rhs=qT_sb[:, h, 0:256],
                        start=True, stop=True)
                    aeAf = WB.tile([128, 128], F32, tag="aeAf", name="aeAf")
                    nc.scalar.activation(aeAf, ps[:, 0:128], AF.Exp,
                                         scale=invs_sb[:, 0:1])
                    nc.vector.tensor_mul(
                        aeA_rb[0][:, h, :], aeAf, expA0_sb[:, h, :])
                    return None
                km = 128 if j < 8 else 16
                qlo = 128 * (j - 1) if j < 8 else 128 * (j - 2)
                bcol = 96 if j < 8 else 224   # B-part column offset
                ps = PSC.tile([128, 256], F32, tag="sc", name="psj")
                nc.tensor.matmul(
                    ps[:km, :],
                    lhsT=kT_sb[:, h, 128 * j:128 * j + km],
                    rhs=qT_sb[:, h, qlo:qlo + 256],
                    start=True, stop=True)
                if j < 8:
                    # A-part of tile j (cols 128:256)
                    aeAf = WB.tile([128, 128], F32, tag="aeAf", name="aeAf")
                    nc.scalar.activation(aeAf, ps[:, 128:256], AF.Exp,
                                         scale=invs_sb[:, j:j + 1])
                    if h % 2 == 0:
                        nc.vector.tensor_mul(
                            aeA_rb[j % 2][:, h, :], aeAf, expA_sb[:, h, :])
                    else:
                        nc.gpsimd.tensor_mul(
                            aeA_rb[j % 2][:, h, :], aeAf, expA_sb[:, h, :])
                # B-part of tile j-1 (rows 0:16, 32 query cols)
                aeBf = WB.tile([16, 32], F32, tag="aeBf", name="aeBf")
                nc.scalar.activation(aeBf, ps[0:16, bcol:bcol + 32],
                                     AF.Exp, scale=invs_sb[0:16, j:j + 1])
                aeB = aeB_rb[h % 4]
                nc.vector.tensor_mul(
                    aeB[:, 96:128], aeBf, expB_sb[:, h, :])
                return aeB

            for h in range(H):
                emit_block_scores(0, h)

            def finish_head(t, h, ao, po, aeB, off):
                hv = h if h < H - 1 else h - 1
                nc.tensor.matmul(po, lhsT=aeB,
                                 rhs=v_sb[0:16, t + 1, hv:hv + 2, :],
                                 start=False, stop=True)
                rinv = WB.tile([128, 1], F32, tag="ri", name="ri")
                nc.vector.reciprocal(rinv, po[:, off + D:off + D + 1])
                nc.vector.tensor_mul(ao[:, 128 * h:128 * h + 128],
                                     po[:, off:off + D],
                                     rinv.to_broadcast([128, D]))

            ao_ring = [None, None, None]  # bf16 attn out, tiles t..t-2
            aoT_ring = [None] * 4     # bf16 transposed (xbar DMA)
            pw_cur = [None]

            def emit_wo(tw, h):
                """Two wo accumulation matmuls for tile tw at step h,
                plus the half copies/DMA at half boundaries."""
                aoT = aoT_ring[tw % 4]
                half = h // 4
                for k in (2 * h % 8, 2 * h % 8 + 1):
                    if k == 0:
                        pw_cur[0] = PWO.tile([128, QW], F32, tag="wo", name="pw")
                    nc.tensor.matmul(
                        pw_cur[0],
                        lhsT=aoT[:, k, :],
                        rhs=woT_sb[:, k, QW * half:QW * half + QW],
                        start=(k == 0), stop=(k == NK - 1))
                    if k == NK - 1:
                        outt = WB2.tile([128, QW], F32, tag="outt")
                        if half == 0:
                            nc.scalar.activation(outt, pw_cur[0], AF.Copy)
                        else:
                            nc.vector.tensor_copy(outt, pw_cur[0])
                        nc.sync.dma_start(
                            out=out_d[128 * tw:128 * tw + 128,
                                      QW * half:QW * half + QW],
                            in_=outt)

            for t in range(NT + 3):
                attn = t < NT
                if attn:
                    ao = WB2.tile([128, DIM], BF16, tag="ao", bufs=3)
                    ao_ring[t % 3] = ao
                    aoT_ring[t % 4] = WB2.tile(
                        [128, NK, 128], BF16, tag="aoT", name="aoT",
                        bufs=4)
                pend = []
                for h in range(H):
                    if h == 4 and attn and t <= NKB - 3:
                        emit_v_block(t + 2)
                    if attn:
                        aeB = emit_block_scores(t + 1, h)
                        hv = h if h < H - 1 else h - 1
                        off = 0 if h < H - 1 else D + 2
                        po = PAT.tile([128, 2 * (D + 2)], F32, tag="at")
                        nc.tensor.matmul(po, lhsT=aeA_rb[t % 2][:, h, :],
                                         rhs=v_sb[:, t, hv:hv + 2, :],
                                         start=True, stop=False)
                        pend.append((h, po, aeB, off))
                        if len(pend) > 2:
                            a = pend.pop(0)
                            finish_head(t, a[0], ao, a[1], a[2], a[3])
                    if 3 <= t:
                        emit_wo(t - 3, h)
                for a in pend:
                    finish_head(t, a[0], ao, a[1], a[2], a[3])
                if attn:
                    # transpose ao(t) via the DMA xbar on the scalar
                    # queue -- never behind the 256KB output writes on
                    # sync (consumed by emit_wo at iteration t+3)
                    nc.scalar.dma_start(out=aoT_ring[t % 4], in_=ao,
                                        transpose=True)

    nc.compile()
    return nc


def _round_f32r(x):
    """RNE to the PE's fp32r grid (11 mantissa bits) -- bit-exact match
    of the engine-side rounding, verified on hardware."""
    u = np.ascontiguousarray(x, np.float32).view(np.uint32).astype(np.uint64)
    half = np.uint64(1 << 11)
    one = np.uint64(1)
    lsb = (u >> np.uint64(12)) & one
    r = ((u + half - one + lsb) & np.uint64(0xFFFFF000)).astype(np.uint32)
    return r.view(np.float32)


def _host_tables():
    slopes = 2.0 ** (-np.arange(1, H + 1, dtype=np.float64))  # [H]
    # A block: keys p (=query start + p), queries n; valid iff 0 <= p-n <= 16
    p = np.arange(128)[:, None]
    n = np.arange(128)[None, :]
    rel = (p - n - 16).astype(np.float64)            # j - i
    validA = (p - n >= 0) & (p - n <= 16)
    expA = np.where(validA[None], np.exp(slopes[:, None, None] * rel[None]), 0.0)
    expA = np.ascontiguousarray(
        expA.transpose(1, 0, 2).astype(np.float32))   # [128, H, 128]
    # B block: keys p' (key idx 128+p'), queries n'' (query 96+n'')
    pp = np.arange(16)[:, None]
    nn = np.arange(32)[None, :]
    relB = (16 + pp - nn).astype(np.float64)
    validB = nn - pp >= 16
    expB = np.where(validB[None], np.exp(slopes[:, None, None] * relB[None]), 0.0)
    expB = np.ascontiguousarray(
        expB.transpose(1, 0, 2).astype(np.float32))   # [16, H, 32]
    # expA with the halo-pad keys (rows 0:16) masked out, for tile 0 on
    # the sequence-start cores (half == 0)
    expA0 = expA.copy()
    expA0[0:16] = 0.0
    return expA, expA0, expB


_CACHE = {}


def make_in_maps(x, wq, wk, wv, wo, q_norm_w, k_norm_w):
    x = np.asarray(x, np.float32)
    expA, expA0, expB = _host_tables()
    BF = ml_dtypes.bfloat16
    # q/k norm weights are ones per the problem spec; fold them into the
    # projection weights (exact when they are ones).
    qnw = np.asarray(q_norm_w, np.float32)
    knw = np.asarray(k_norm_w, np.float32)
    wqT = np.ascontiguousarray(
        np.asarray(wq, np.float32).T * qnw[None, :]).astype(BF)
    wkT = np.ascontiguousarray(
        np.asarray(wk, np.float32).T * knw[None, :]).astype(BF)
    wvT = np.ascontiguousarray(np.asarray(wv, np.float32).T).astype(BF)
    woT = np.ascontiguousarray(np.asarray(wo, np.float32).T).astype(BF)

    in_maps = []
    for c in range(N_CORES):
        b, half = c // 2, c % 2
        start = half * T
        if half == 0:
            chunk = np.concatenate(
                [np.zeros((HALO, DIM), np.float32), x[b, 0:T]], axis=0)
        else:
            chunk = x[b, start - HALO:start + T]
        xT_c = np.ascontiguousarray(chunk.T).astype(BF)  # [dim, TK]
        in_maps.append({
            "xT": xT_c, "wqT": wqT, "wkT": wkT, "wvT": wvT, "woT": woT,
            "expA": expA, "expA0": expA0 if half == 0 else expA,
            "expB": expB,
        })
    return in_maps


def assemble_out(results):
    out = np.empty((4, 2048, DIM), np.float32)
    for c in range(N_CORES):
        b, half = c // 2, c % 2
        out[b, half * T:half * T + T] = results[c]["out"]
    return out


def get_nc():
    if "nc" not in _CACHE:
        _CACHE["nc"] = _build_nc()
    return _CACHE["nc"]


def kernel(x, wq, wk, wv, wo, q_norm_w, k_norm_w):
    assert x.shape == (4, 2048, 1024)
    nc = get_nc()
    in_maps = make_in_maps(x, wq, wk, wv, wo, q_norm_w, k_norm_w)
    res = run_bass_kernel_spmd(nc, in_maps, list(range(N_CORES)))
    return assemble_out(res.results)

